# revision 8
# baseline (speedup 1.0000x reference)
"""DenseCRF mean-field inference on 8 Trainium2 NeuronCores.

Math per image: 5 iterations of
    q_hat = U + 4*((q/n) @ K)/n + 2*conv71(q);  q = softmax(q_hat, axis=0)
with K[i,j] = exp(-0.5*d2(i,j)) the dense 9216x9216 bilateral kernel and
n = sqrt(colsum K).  Columns of K are sharded over the 8 cores.

Design:
  * Resident matrix is the UNRESCALED E = 4K in fp8e4 (10.6 MB SBUF/core).
    Built by a 9-row bf16 extended-feature matmul (features bf16-rounded on
    host; diagonal terms computed from the rounded features and carried as
    hi/lo bf16 row pairs on both sides, so the quadratic form cancels to
    ~1e-2) + ACT Exp over 1536-element flat windows (chunk-agnostic since
    no per-chunk bias remains), double-buffered in 3-bank PSUM tiles.
    The build is ACT-bound (~79us); E-matmuls, the q0 AllGather, and the
    colsum ranges A/B hide under it.
  * rn = 1/sqrt(colsum K): local fp8-DoubleRow ones-matvec colsums
    (ranges A/B accumulate inside the build in spare PSUM banks), then a
    bf16 AllGather of per-core colsums (= rowsums by symmetry) gives rn
    for all pixels; rn_j for my columns stays local/f32.
  * Per iteration: rn_i folds into a fp8 lhsT q-scale (one DVE mult);
    the matvec runs in fp8 DoubleRow (2 k-tiles/instr, 0.5 cyc/row,
    lhsT padded 21->32 cols: walrus requires M % 32 == 0), range-major
    over 3 PSUM column splits so the rn_j scale of early columns
    overlaps later accumulation.  The contraction order is permuted
    (chunk n, PE row r <-> pixel 72r+n) so the post-AllGather lhsT
    gather is one contiguous 1512B run per partition.
  * The 2*conv71 spatial term is separable: per y-row block via tiny
    band-matrix matmuls (By[:,mine]^T @ Q then 2Bx), and is folded --
    together with the pixel-major unary U -- into the per-y-row
    transpose PSUM groups via identity-matmul accumulation, so softmax
    needs no extra elementwise adds.
  * Softmax per y-row in transposed [96,21] tiles (one batched Exp +
    free-axis reduce + reciprocal scale); q shards AllGather in fp8
    (out 194KB, ~20us; the 15us fixed collective cost dominates).
    Dummy f32 matmuls into a scrap PSUM bank keep the PE p-state warm
    across each collective.
"""

import numpy as np
import ml_dtypes

H = 96
W = 96
P = H * W            # 9216 pixels
L = 21               # classes
NCORES = 8
PSH = P // NCORES    # 1152 pixels per core
NI = P // 128        # 72 contraction chunks of 128
NJ = PSH // 128      # 9 (kept for test.py compat)
NR = H // NCORES     # 12 y-rows per core
NPAIR = NI // 2      # 36 DoubleRow pairs
SXY_BF = 70.0
SC_BF = 12.0
SIG_SQ_SP = 36.0
RR_SP = 35
LN4 = float(np.log(4.0))
LN2 = float(np.log(2.0))

_bf16 = ml_dtypes.bfloat16
_fp8 = ml_dtypes.float8_e4m3

_CACHE = {}
TRACE = False
LAST_RESULT = None
NITERS = 5


# ----------------------------------------------------------------------------
# host-side prep
# ----------------------------------------------------------------------------

def _spatial_band():
    if "band" in _CACHE:
        return _CACHE["band"]
    g1 = np.exp(-((np.arange(2 * RR_SP + 1, dtype=np.float64) - RR_SP) ** 2)
                / (2 * SIG_SQ_SP))
    z = g1.sum()
    idx = np.arange(H)
    d = idx[:, None] - idx[None, :]
    B = np.where(np.abs(d) <= RR_SP,
                 np.exp(-(d.astype(np.float64) ** 2) / (2 * SIG_SQ_SP)) / z,
                 0.0).astype(np.float32)
    _CACHE["band"] = B
    return B


def _in_maps(unary, ref):
    B = _spatial_band()
    u = np.asarray(unary, np.float32).reshape(L, P)
    r = np.asarray(ref, np.float32).reshape(3, P)

    ys = (np.arange(P) // W).astype(np.float32)
    xs = (np.arange(P) % W).astype(np.float32)
    f = np.concatenate([ys[None] / SXY_BF, xs[None] / SXY_BF, r / SC_BF], 0)
    fhat = f.astype(_bf16)                       # [5, P] rounded features
    fh32 = fhat.astype(np.float32)
    d = (-0.5 * (fh32 * fh32).sum(0))            # [P] f32, exact from rounded
    dhi = d.astype(_bf16)
    dlo = (d - dhi.astype(np.float32)).astype(_bf16)

    # contraction permutation: chunk n, PE row r <-> pixel 72*r + n, so the
    # per-iteration lhsT gather reads one contiguous 1512B run per partition
    perm = (np.arange(P).reshape(128, NI).T).ravel()   # pos 128*n+r -> 72r+n
    dl4 = d + LN4
    dl4hi = dl4.astype(_bf16)
    dl4lo = (dl4 - dl4hi.astype(np.float32)).astype(_bf16)
    ltP = np.concatenate([fhat, dl4hi[None], dl4lo[None],
                          np.ones((2, P), _bf16)], 0)[:, perm]  # [9, P]

    uc = np.clip(u, 1e-5, 1.0)                   # [L, P]
    Ufull = np.log(uc)
    q0 = uc / uc.sum(0, keepdims=True)           # [L, P]
    q0pm = np.ascontiguousarray(q0.T.astype(_fp8))  # [P, L] pixel-major

    maps = []
    for c in range(NCORES):
        sl = slice(c * PSH, (c + 1) * PSH)
        qrQ = np.concatenate([fhat[:, sl], np.ones((2, PSH), _bf16),
                              dhi[None, sl], dlo[None, sl]], 0)
        byM = np.ascontiguousarray(B[c * NR:(c + 1) * NR, :].T.astype(_bf16))
        maps.append({
            "ltP": np.ascontiguousarray(ltP),
            "qrQ": np.ascontiguousarray(qrQ),
            "upm": np.ascontiguousarray(
                Ufull[:, sl].T.reshape(NR, W, L).transpose(1, 0, 2)),
            "q0sh": np.ascontiguousarray(q0pm[sl].reshape(NR, W, L)),
            "byM": byM,                                     # [96, 12]
            "bx2": np.ascontiguousarray((2.0 * B).astype(_bf16)),  # [96, 96]
        })
    return maps


# ----------------------------------------------------------------------------
# device program
# ----------------------------------------------------------------------------

def _build_bass(niters=NITERS):
    key = ("nc2", niters)
    if key in _CACHE:
        return _CACHE[key]

    import concourse.bass as bass
    import concourse.bacc as bacc
    import concourse.tile as tile
    import concourse.mybir as mybir
    from concourse.masks import make_identity

    f32 = mybir.dt.float32
    bf16 = mybir.dt.bfloat16
    fp8 = mybir.dt.float8e4
    AF = mybir.ActivationFunctionType
    ALU = mybir.AluOpType
    DR = mybir.MatmulPerfMode.DoubleRow

    nc = bacc.Bacc("TRN2", num_devices=NCORES)

    ltP = nc.dram_tensor("ltP", [9, P], bf16, kind="ExternalInput")
    qrQ = nc.dram_tensor("qrQ", [9, PSH], bf16, kind="ExternalInput")
    upm = nc.dram_tensor("upm", [W, NR, L], f32, kind="ExternalInput")
    q0sh = nc.dram_tensor("q0sh", [NR, W, L], fp8, kind="ExternalInput")
    byM = nc.dram_tensor("byM", [H, NR], bf16, kind="ExternalInput")
    bx2 = nc.dram_tensor("bx2", [W, W], bf16, kind="ExternalInput")
    qout = nc.dram_tensor("qout", [NR, W, L], f32, kind="ExternalOutput")

    rg = [list(range(NCORES))]
    COLS = ((0, 512), (512, 512), (1024, 128))

    with tile.TileContext(nc) as tc:
        with tc.tile_pool(name="dram", bufs=1, space="DRAM") as dram:
            qsh_d = dram.tile([PSH * L], fp8)       # my q shard (r x l)
            qfl_d = dram.tile([P * L], fp8)         # gathered q
            cs_in_d = dram.tile([PSH], bf16)        # my colsums (pixel order)
            rnj_d = dram.tile([1, PSH], f32)        # my rn_j free-major
            cs_out_d = dram.tile([P], bf16)         # all colsums

            qfl_lhs = qfl_d.rearrange("(p n l) -> p n l", p=128, n=NI, l=L)
            qfl_y = qfl_d.rearrange("(y x l) -> y x l", y=H, x=W, l=L)

            with tc.tile_pool(name="persist", bufs=1) as persist:
                # pin the ACT table to the combined Ln+Exp set up front so
                # the greedy insert_act_table_loads pass never ping-pongs
                # between the Ln-only and Exp-only sets (5x 1.28us saved)
                from concourse.hw_specs import get_activation_tables
                _sets = list(get_activation_tables(nc.m.arch).keys())
                nc.scalar.add_instruction(mybir.InstLoadActFuncSet(
                    name=nc.get_next_instruction_name(),
                    act_func_set_id=_sets.index("natural_log_exp_and_others"),
                    ins=[], outs=[]))
                ident = persist.tile([L, L], f32)
                make_identity(nc, ident[:])
                ltP_sb = persist.tile([9, P], bf16)
                nc.sync.dma_start(ltP_sb[:], ltP[:, :])
                qrQ_sb = persist.tile([9, PSH], bf16)
                nc.sync.dma_start(qrQ_sb[:], qrQ[:, :])
                byM_sb = persist.tile([H, NR], bf16)
                nc.sync.dma_start(byM_sb[:], byM[:, :])
                bx2_sb = persist.tile([W, W], bf16)
                nc.sync.dma_start(bx2_sb[:], bx2[:, :])
                Upm_sb = persist.tile([W, NR, L], f32)
                nc.sync.dma_start(Upm_sb[:], upm[:, :, :])
                ident96b = persist.tile([W, W], bf16)
                make_identity(nc, ident96b[:])
                ident96f = persist.tile([W, W], f32)
                make_identity(nc, ident96f[:])

                Mt = persist.tile([128, NI, PSH], fp8, name="Mt")
                rnJf = persist.tile([1, PSH], f32)
                rnJ21 = persist.tile([L, PSH], f32)
                ones2 = persist.tile([128, 2, 32], fp8)
                nc.vector.memset(ones2[:], 1.0)
                ln2c = persist.tile([128, 1], f32)
                nc.vector.memset(ln2c[:], LN2)
                qs = persist.tile([128, NI, 32], fp8, name="qs")
                nc.vector.memset(qs[:], 0.0)

                # ---- q0 AllGather (overlaps the E build) ------------------
                nc.gpsimd.dma_start(
                    qsh_d.rearrange("(r x l) -> r x l", r=NR, x=W, l=L),
                    q0sh[:, :, :])
                nc.gpsimd.collective_compute(
                    "AllGather", mybir.AluOpType.bypass, replica_groups=rg,
                    ins=[qsh_d[:]], outs=[qfl_d[:]])

                # ---- E = exp(T') build + local colsum ---------------------
                csb = persist.tile([1, PSH], f32)
                MtF = Mt[:].rearrange("p a b -> p (a b)")
                WIN = 1536
                NW = NI * PSH // WIN
                with (
                    tc.tile_pool(name="eps", bufs=2, space="PSUM") as eps,
                    tc.tile_pool(name="csp", bufs=1, space="PSUM") as csp,
                ):
                    csA = csp.tile([32, 512], f32, name="csA")
                    csB = csp.tile([32, 512], f32, name="csB")
                    kcs = 0
                    for w in range(NW):
                        ps = eps.tile([128, WIN], f32, tag="eps")
                        a = WIN * w
                        for n in range(a // PSH, (a + WIN - 1) // PSH + 1):
                            lo = max(a, PSH * n)
                            hi = min(a + WIN, PSH * (n + 1))
                            st = lo
                            while st < hi:
                                en = min(hi,
                                         a + ((st - a) // 512 + 1) * 512)
                                nc.tensor.matmul(
                                    ps[:, st - a:en - a],
                                    ltP_sb[:, 128 * n:128 * (n + 1)],
                                    qrQ_sb[:, st - PSH * n:en - PSH * n],
                                    start=True, stop=True)
                                st = en
                        nc.scalar.activation(MtF[:, a:a + WIN], ps[:],
                                             AF.Exp)
                        # colsum pairs whose chunks are fully expo'd
                        while (kcs + 1) * 2 * PSH <= a + WIN:
                            for cs_t, o in ((csA, 0), (csB, 512)):
                                nc.tensor.matmul(
                                    cs_t[:], ones2[:],
                                    Mt[:, 2 * kcs:2 * kcs + 2, o:o + 512],
                                    start=(kcs == 0),
                                    stop=(kcs == NPAIR - 1),
                                    perf_mode=DR, skip_group_check=True)
                            kcs += 1
                    nc.vector.tensor_copy(csb[0:1, 0:512], csA[0:1, :])
                    nc.vector.tensor_copy(csb[0:1, 512:1024], csB[0:1, :])

                # colsum range C (post-build) + rn_j
                with tc.tile_pool(name="cspC", bufs=1, space="PSUM") as cspC:
                    csC = cspC.tile([32, 128], f32, name="csC")
                    for k in range(NPAIR):
                        nc.tensor.matmul(
                            csC[:], ones2[:],
                            Mt[:, 2 * k:2 * k + 2, 1024:1152],
                            start=(k == 0), stop=(k == NPAIR - 1),
                            perf_mode=DR)
                    nc.vector.tensor_copy(csb[0:1, 1024:1152], csC[0:1, :])
                    csbh = persist.tile([1, PSH], bf16)
                    nc.vector.tensor_copy(csbh[:], csb[:])
                    nc.sync.dma_start(
                        cs_in_d.rearrange("(a q) -> a q", a=1, q=PSH),
                        csbh[:])

                # ---- rn_i for all pixels via colsum AllGather; the local
                # rn_j chain (Ln/Exp + table loads) hides under it ----------
                nc.gpsimd.collective_compute(
                    "AllGather", mybir.AluOpType.bypass, replica_groups=rg,
                    ins=[cs_in_d[:]], outs=[cs_out_d[:]])
                with tc.tile_pool(name="rjp", bufs=1) as rjp:
                    lcj = rjp.tile([1, PSH], f32)
                    nc.scalar.activation(lcj[:], csb[0:1, :], AF.Ln)
                    nc.scalar.activation(rnJf[:], lcj[:], AF.Exp,
                                         bias=ln2c[0:1, :], scale=-0.5)
                    nc.sync.dma_start(rnj_d[:, :], rnJf[:])
                    nc.sync.dma_start(
                        rnJ21[:], rnj_d[0:1, :].to_broadcast((L, PSH)))
                csg = persist.tile([128, NI], bf16)
                nc.sync.dma_start(
                    csg[:], cs_out_d.rearrange("(p n) -> p n", p=128, n=NI))
                lci = persist.tile([128, NI], f32)
                nc.scalar.activation(lci[:], csg[:], AF.Ln)
                rnf = persist.tile([128, NI], f32)
                nc.scalar.activation(rnf[:], lci[:], AF.Exp,
                                     bias=ln2c[:, :], scale=-0.5)


                # ---- iterations ------------------------------------------
                with (
                    tc.tile_pool(name="itq", bufs=1) as itq,
                    tc.tile_pool(name="mmp", bufs=1, space="PSUM") as mmp,
                    tc.tile_pool(name="spp", bufs=1, space="PSUM") as spp,
                    tc.tile_pool(name="tpp", bufs=1, space="PSUM") as tpp,
                    tc.tile_pool(name="wmp", bufs=1, space="PSUM") as wmp,
                ):
                    def pe_warm(nwm):
                        # keep the PE p-state hot through a collective:
                        # f32 matmuls (4 cyc/row) into a scrap bank
                        wt = wmp.tile([W, NR, L], f32, tag="warm")
                        for _ in range(nwm):
                            nc.tensor.matmul(wt[:], ident96f[:],
                                             Upm_sb[:, :, :],
                                             start=True, stop=True)

                    pe_warm(26)
                    for it in range(1, niters + 1):
                        qf = itq.tile([128, NI, L], fp8, tag="qf", bufs=2)
                        nc.sync.dma_start(qf[:], qfl_lhs)
                        qy = itq.tile([H, W, L], fp8, tag="qy", bufs=2)
                        nc.scalar.dma_start(qy[:], qfl_y)

                        # lhsT = q * rn_i in fp8 (pad cols 21:32 stay 0)
                        nc.vector.tensor_mul(
                            qs[:, 0:NI // 2, 0:L], qf[:, 0:NI // 2, :],
                            rnf[:, 0:NI // 2].to_broadcast((128, NI // 2, L)))
                        nc.vector.tensor_mul(
                            qs[:, NI // 2:, 0:L], qf[:, NI // 2:, :],
                            rnf[:, NI // 2:].to_broadcast((128, NI // 2, L)))

                        # spatial: tmpY = By_mine^T @ Qy  -> [12, (x l)]
                        tmpYs = itq.tile([NR, W, L], f32, tag="tmpYs")
                        for h in range(2):
                            tmpY = spp.tile([NR, 2, 512], f32, tag="tmpY")
                            for g2 in range(2):
                                g = 2 * h + g2
                                nc.tensor.matmul(
                                    tmpY[:, g2, 0:504], byM_sb[:],
                                    qy[:, 24 * g:24 * (g + 1), :],
                                    start=True, stop=True)
                            nc.scalar.copy(tmpYs[:, 48 * h:48 * (h + 1), :],
                                           tmpY[:, :, 0:504])
                        txa = spp.tile([W, L, NR], f32, tag="txa")
                        for ll in range(L):
                            nc.tensor.transpose(txa[:, ll, :],
                                                tmpYs[:, :, ll],
                                                ident[0:NR, 0:NR])
                        txs = itq.tile([W, L, NR], bf16, tag="txs")
                        nc.scalar.copy(txs[:], txa[:])
                        qsfT = spp.tile([W, L, NR], f32, tag="txa")
                        nc.tensor.matmul(qsfT[:], bx2_sb[:], txs[:],
                                         start=True, stop=True)
                        qsfS = itq.tile([W, L, NR], bf16, tag="qsfS")
                        nc.scalar.copy(qsfS[:], qsfT[:])

                        # bilateral: ps = (q rn) @ E, fp8 DoubleRow.
                        # Range-major so qh/softmax of early columns overlap
                        # the later ranges' accumulation.
                        ps = mmp.tile([32, PSH], f32, tag="ps")
                        qh = itq.tile([L, PSH], f32, tag="qh")
                        for (o, n) in COLS:
                            for k in range(NPAIR):
                                nc.tensor.matmul(
                                    ps[:, o:o + n], qs[:, 2 * k:2 * k + 2, :],
                                    Mt[:, 2 * k:2 * k + 2, o:o + n],
                                    start=(k == 0), stop=(k == NPAIR - 1),
                                    perf_mode=DR)
                            nc.vector.tensor_mul(qh[:, o:o + n],
                                                 ps[0:L, o:o + n],
                                                 rnJ21[:, o:o + n])

                        # per-y-row transpose; spatial + U folded in via
                        # identity-matmul accumulation on the PE
                        tp = tpp.tile([W, NR, L], f32, tag="tp")
                        for r in range(NR):
                            nc.tensor.matmul(
                                tp[:, r, :], qh[:, r * W:(r + 1) * W],
                                ident[:], is_transpose=True,
                                start=True, stop=False)
                            nc.tensor.matmul(
                                tp[:, r, :], ident96b[:], qsfS[:, :, r],
                                start=False, stop=False)
                            nc.tensor.matmul(
                                tp[:, r, :], ident96f[:], Upm_sb[:, r, :],
                                start=False, stop=True)
                        e = itq.tile([W, NR, L], f32, tag="e")
                        nc.scalar.activation(e[:], tp[:], AF.Exp)
                        zz = itq.tile([W, NR], f32, tag="zz")
                        nc.vector.tensor_reduce(zz[:], e[:],
                                                mybir.AxisListType.X,
                                                ALU.add)
                        rz = itq.tile([W, NR], f32, tag="rz")
                        nc.vector.reciprocal(rz[:], zz[:])
                        if it < niters:
                            qm = itq.tile([W, NR, L], fp8, tag="qm")
                            nc.vector.tensor_mul(
                                qm[:], e[:],
                                rz[:].to_broadcast((W, NR, L)))
                            nc.sync.dma_start(
                                qsh_d.rearrange("(r x l) -> x r l",
                                                r=NR, x=W, l=L), qm[:])
                            nc.gpsimd.collective_compute(
                                "AllGather", mybir.AluOpType.bypass,
                                replica_groups=rg,
                                ins=[qsh_d[:]], outs=[qfl_d[:]])
                            pe_warm(34)
                        else:
                            qo = itq.tile([W, NR, L], f32, tag="qo")
                            nc.vector.tensor_mul(
                                qo[:], e[:],
                                rz[:].to_broadcast((W, NR, L)))
                            nc.sync.dma_start(
                                qout[:, :, :].rearrange("r x l -> x r l"),
                                qo[:])

    nc.finalize()
    _CACHE[key] = nc
    return nc


# ----------------------------------------------------------------------------
# host entry point
# ----------------------------------------------------------------------------

def kernel(unary: np.ndarray, ref: np.ndarray) -> np.ndarray:
    from concourse import bass_utils

    nc = _build_bass()
    in_maps = _in_maps(unary, ref)

    global LAST_RESULT
    res = bass_utils.run_bass_kernel_spmd(nc, in_maps,
                                          core_ids=list(range(NCORES)),
                                          trace=TRACE)
    LAST_RESULT = res
    shards = [res.results[c]["qout"].reshape(PSH, L) for c in range(NCORES)]
    qfull = np.concatenate(shards, 0)          # [P, L]
    out = qfull.T.reshape(1, L, H, W).astype(np.float32)
    return out


if __name__ == "__main__":
    u = np.random.rand(1, L, H, W).astype(np.float32)
    r = (np.random.rand(1, 3, H, W) * 255).astype(np.float32)
    o = kernel(u, r)
    print(o.shape, o.dtype, o.sum())
